# revision 1
# baseline (speedup 1.0000x reference)
"""Trainium2 Bass kernel for nn_ButterflyFilter.

The reference applies, per length-512 row (flattened b*c*angles):
  zero-pad to 1024 -> 10-stage butterfly "FFT" (stage order decreasing)
  -> elementwise filter (bit-reversed order) -> 10-stage butterfly
  "IFFT" (stage order increasing) -> real part of first 512 entries.

Every step is linear in x, so the whole chain is one complex 1024x1024
operator A determined by (twiddle_fft, twiddle_ifft, fourier_filter_br).
Since x is real with support on [:512] and only Re(y)[:512] is kept, the
effective map is the real 512x512 matrix W = Re(A)[:512, :512]:

    proj_row = W @ x_row

x in HBM is (b, c, s, a) — for fixed (b, c) the tile is (s, a), i.e. rows
(angles) are already laid out column-major, exactly the moving-operand
layout the TensorEngine wants. So the device work is 16 independent
512x512x512 matmuls out_bc = W @ x_bc, data-parallel 2 per core across
8 cores. The small parameter-folding (building W from the twiddles) runs
on host in float64; the 32 MiB of row data never touches the host math.
"""

import os
import sys
import types
from contextlib import ExitStack

import numpy as np

import concourse.bass as bass
import concourse.mybir as mybir
from concourse.bass_utils import run_bass_kernel_spmd


def _ensure_axon_hooks():
    # concourse.bass_utils imports antenv.axon_hooks on the trace path; some
    # images lack that module. Provide a no-op holder so a BASS_TRACE env set
    # by the caller can't crash the run.
    try:
        import antenv.axon_hooks  # noqa: F401
    except Exception:
        m = types.ModuleType("antenv.axon_hooks")
        m._h = None
        m.set_axon_ntff_profile_hook = lambda h: setattr(m, "_h", h)
        m.get_axon_ntff_profile_hook = lambda: m._h
        sys.modules["antenv.axon_hooks"] = m


_ensure_axon_hooks()

N_CORES = 8
S = 512          # input/output row length
NF = 1024        # padded length
P = 128          # SBUF partitions
BC_PER_CORE = 2  # 16 (b,c) tiles / 8 cores

# Exposed for the test harness: exec time of the last device run (ns), if
# profiling was enabled via BUTTERFLY_TRACE=1.
last_exec_time_ns = None
last_results = None


def _butterfly_np(tw, x, increasing):
    # Mirrors the reference butterfly exactly, in numpy (any dtype).
    B, n = x.shape
    m = tw.shape[0]
    order = range(m) if increasing else range(m - 1, -1, -1)
    for idx in order:
        s = 1 << idx
        t = tw[idx].reshape(n // (2 * s), s, 2, 2)
        xr = x.reshape(B, n // (2 * s), 2, s)
        x = np.einsum('gjik,bgkj->bgij', t, xr).reshape(B, n)
    return x


def _compose_wt(twiddle_fft, twiddle_ifft, fourier_filter_br):
    """Fold twiddles+filter into the lhsT operand Wt[i_in, o_out] (512x512 f32)."""
    tw_fft = np.asarray(twiddle_fft, dtype=np.float64)
    tw_ifft = np.asarray(twiddle_ifft, dtype=np.float64)
    filt = np.asarray(fourier_filter_br, dtype=np.float64)
    tf = tw_fft[0, ..., 0] + 1j * tw_fft[0, ..., 1]
    ti = tw_ifft[0, ..., 0] + 1j * tw_ifft[0, ..., 1]
    X = np.eye(NF, dtype=np.complex128)      # row j = e_j
    X = _butterfly_np(tf, X, increasing=False)
    X = X * filt[None, :]
    X = _butterfly_np(ti, X, increasing=True)
    # X = chain(I) = A^T, so X[i, o] = A[o, i]; W[o, i] = Re(A[o, i]).
    # lhsT for out = lhsT.T @ rhs must be Wt[i, o] = W[o, i] = Re(X[i, o]).
    return np.ascontiguousarray(np.real(X[:S, :S]).astype(np.float32))


def _mm_dtype():
    return (
        mybir.dt.float32r
        if os.environ.get("BUTTERFLY_MM_DTYPE", "fp32r") == "fp32r"
        else mybir.dt.float32
    )


def _build_nc():
    # Raw Bass (no TileContext): this walrus encodes at most ONE semaphore
    # wait per instruction, which Tile's scheduler and epilogue drain cannot
    # guarantee. With manual engine programs every wait is its own wait_ge.
    #
    # Layout (per core):
    #   wx[k] (128, 1024) = [W_k | x0_k]: contraction chunk k of the operator
    #   fused with bc-tile-0's chunk, one 512 KiB DMA piece each, so compute
    #   starts on the first piece. x1[k] (128, 512) are bc-tile-1's chunks.
    #   out_bc[o*128+p, a] accumulates in one PSUM bank per (bc, o) group,
    #   is copied to SBUF by DVE, and stored as 256 KiB contiguous chunks.
    mmdt = _mm_dtype()
    kc = S // P  # 4 contraction chunks
    oc = S // P  # 4 output-row chunks
    f32 = mybir.dt.float32
    # PE warm-up matmuls (HAM un-throttle) during the input DMA wait. Each
    # fp32 matmul emits 2 HW passes at ~640 ns cold, so 3 calls ~= 3.8 us of
    # dense PE busy — enough to trip HAM's ~3.4 us SHORT window right as the
    # first input piece lands (measured: 2 calls leave the real stream cold).
    n_warm = 3

    nc = bass.Bass()
    wx = nc.declare_dram_parameter("wx", [kc, P, 2 * S], mmdt, isOutput=False)
    x1d = nc.declare_dram_parameter("x1", [kc, P, S], mmdt, isOutput=False)
    out = nc.declare_dram_parameter("out", [BC_PER_CORE, S, S], f32, isOutput=True)

    with ExitStack() as ctx:
        wx_sb = [
            ctx.enter_context(nc.sbuf_tensor(f"wx_sb{k}", [P, 2 * S], mmdt))
            for k in range(kc)
        ]
        x1_sb = ctx.enter_context(nc.sbuf_tensor("x1_sb", [P, 4 * S], mmdt))
        warm_sb = ctx.enter_context(nc.sbuf_tensor("warm_sb", [P, 3 * P + 32], f32))
        o_sb = [
            ctx.enter_context(nc.sbuf_tensor(f"o_sb{j}", [P, 4 * S], f32))
            for j in range(2)
        ]
        accs = [
            ctx.enter_context(nc.psum_tensor(f"acc{g}", [P, S], f32))
            for g in range(BC_PER_CORE * oc)
        ]
        s_wx = [ctx.enter_context(nc.semaphore(f"s_wx{k}")) for k in range(kc)]
        s_x1 = [ctx.enter_context(nc.semaphore(f"s_x1{k}")) for k in range(kc)]
        s_warm = ctx.enter_context(nc.semaphore("s_warm"))
        s_pe = ctx.enter_context(nc.semaphore("s_pe"))
        s_dve = ctx.enter_context(nc.semaphore("s_dve"))
        s_cpa = ctx.enter_context(nc.semaphore("s_cpa"))
        s_out = ctx.enter_context(nc.semaphore("s_out"))
        block = ctx.enter_context(nc.Block())

        @block.sync
        def _(sync):
            # Input pieces, issue order = consumption order. 512 KiB each for
            # wx (W chunk fused with bc0 x chunk), 256 KiB each for x1.
            for k in range(kc):
                sync.dma_start(wx_sb[k][:], wx[k]).then_inc(s_wx[k], 16)
            for k in range(kc):
                sync.dma_start(x1_sb[:, bass.ts(k, S)], x1d[k]).then_inc(s_x1[k], 16)
            sync.wait_ge(s_out, BC_PER_CORE * oc * 16)

        @block.tensor
        def _(tensor):
            # Warm-up matmuls on a zeroed scratch tile: keeps the PE busy
            # while inputs stream in so HAM un-throttles (1.2 -> 2.4 GHz)
            # before the real matmuls. Results land in acc 7 which is cleared
            # by its real accumulation group's start=True much later.
            tensor.wait_ge(s_warm, 1)
            for _ in range(n_warm):
                nc.tensor.matmul(
                    accs[-1][:, : 2 * P], warm_sb[:, :P], warm_sb[:, P : 3 * P],
                    start=True, stop=True,
                )
            # bc0: k-outer so compute starts on the first 512 KiB piece.
            for k in range(kc):
                tensor.wait_ge(s_wx[k], 16)
                for o in range(oc):
                    mm = nc.tensor.matmul(
                        accs[o][:],
                        wx_sb[k][:, bass.ts(o, P)],
                        wx_sb[k][:, S : 2 * S],
                        start=(k == 0),
                        stop=(k == kc - 1),
                    )
                    if k == kc - 1:
                        mm.then_inc(s_pe, 1)
            # bc1
            for k in range(kc):
                tensor.wait_ge(s_x1[k], 16)
                for o in range(oc):
                    mm = nc.tensor.matmul(
                        accs[oc + o][:],
                        wx_sb[k][:, bass.ts(o, P)],
                        x1_sb[:, bass.ts(k, S)],
                        start=(k == 0),
                        stop=(k == kc - 1),
                    )
                    if k == kc - 1:
                        mm.then_inc(s_pe, 1)

        @block.vector
        def _(vector):
            nc.vector.memset(warm_sb[:], 0.0).then_inc(s_warm, 1)
            for g in range(BC_PER_CORE * oc):
                bc, o = divmod(g, oc)
                vector.wait_ge(s_pe, g + 1)
                nc.vector.tensor_copy(
                    o_sb[bc][:, bass.ts(o, S)], accs[g][:]
                ).then_inc(s_dve, 1)

        @block.scalar
        def _(scalar):
            # Per-group 256 KiB stores from the otherwise-idle ACT engine so
            # output drains as soon as each o-chunk is copied out of PSUM.
            for g in range(BC_PER_CORE * oc):
                bc, o = divmod(g, oc)
                scalar.wait_ge(s_dve, g + 1)
                scalar.dma_start(
                    out[bc, bass.ts(o, P), :], o_sb[bc][:, bass.ts(o, S)]
                ).then_inc(s_out, 16)

    return nc


def kernel(x, twiddle_fft, twiddle_ifft, fourier_filter_br):
    global last_exec_time_ns, last_results
    x = np.asarray(x, dtype=np.float32)
    b, c, s_len, a = x.shape
    assert (b, c, s_len, a) == (8, 2, S, S)

    wt = _compose_wt(twiddle_fft, twiddle_ifft, fourier_filter_br)
    x16 = x.reshape(b * c, S // P, P, S)  # [bc, k, p, m]
    wt4 = wt.reshape(S // P, P, S)

    in_maps = []
    for core in range(N_CORES):
        x0 = x16[BC_PER_CORE * core]
        x1 = x16[BC_PER_CORE * core + 1]
        # wx[k] = [w_k | x0_k] along the free dim, one 512 KiB DMA piece each
        wx = np.concatenate([wt4, x0], axis=2)  # (4, 128, 1024)
        in_maps.append(
            {
                "wx": np.ascontiguousarray(wx),
                "x1": np.ascontiguousarray(x1),
            }
        )
    nc = _build_nc()
    trace = os.environ.get("BUTTERFLY_TRACE") == "1"
    res = run_bass_kernel_spmd(nc, in_maps, core_ids=list(range(N_CORES)), trace=trace)
    last_exec_time_ns = res.exec_time_ns
    last_results = res

    q = np.concatenate([res.results[k]["out"] for k in range(N_CORES)], axis=0)
    # q[bc, o, a] = proj.T[o, bc*512 + a]; reference output is
    # proj.T.reshape(b, c, s, a) — a pure reinterpret of the (512, 8192) buffer.
    out = q.transpose(1, 0, 2).reshape(S, b * c * a).reshape(b, c, s_len, a)
    return np.ascontiguousarray(out).astype(np.float32)



# revision 2
# speedup vs baseline: 1.2837x; 1.2837x over previous
"""Trainium2 Bass kernel for nn_ButterflyFilter.

The reference applies, per length-512 row (flattened b*c*angles):
  zero-pad to 1024 -> 10-stage butterfly "FFT" (stage order decreasing)
  -> elementwise filter (bit-reversed order) -> 10-stage butterfly
  "IFFT" (stage order increasing) -> real part of first 512 entries.

Every step is linear in x, so the whole chain is one complex 1024x1024
operator A determined by (twiddle_fft, twiddle_ifft, fourier_filter_br).
Since x is real with support on [:512] and only Re(y)[:512] is kept, the
effective map is the real 512x512 matrix W = Re(A)[:512, :512]:

    proj_row = W @ x_row

x in HBM is (b, c, s, a) — for fixed (b, c) the tile is (s, a), i.e. rows
(angles) are already laid out column-major, exactly the moving-operand
layout the TensorEngine wants. So the device work is 16 independent
512x512x512 matmuls out_bc = W @ x_bc, data-parallel 2 per core across
8 cores. The small parameter-folding (building W from the twiddles) runs
on host in float64; the 32 MiB of row data never touches the host math.

Performance notes (vs the fp32 version):
  * The rel-err gate is 2e-2; fp16 operands + fp32 PSUM accumulation +
    fp16 output store land ~1e-3, so the whole pipeline runs in fp16.
    That halves input DMA (3 MiB -> 1.5 MiB/core) and output DMA
    (2 MiB -> 1 MiB/core). PE pitch at N=512 is one pass for fp32r and
    fp16 alike, so compute time is unchanged — DMA was the bottleneck.
  * Stream order is bc-separated: [W_k|x0_k] fused pieces on the sync
    HWDGE ring, the whole x1 on the scalar ring. bc0's outputs then
    drain while bc1 is still computing; bc1 runs group-sequential
    (o-outer) so its four output chunks stagger into the store queue
    instead of bunching at the end.
  * Block(no_gpsimd_drain=True) skips the expensive gpsimd dge_drain in
    the exit barrier (this kernel issues no SWDGE DMAs).
"""

import os
import sys
import types
from contextlib import ExitStack

import numpy as np

import concourse.bass as bass
import concourse.mybir as mybir
from concourse.bass_utils import run_bass_kernel_spmd


def _ensure_axon_hooks():
    # concourse.bass_utils imports antenv.axon_hooks on the trace path; some
    # images lack that module. Provide a no-op holder so a BASS_TRACE env set
    # by the caller can't crash the run.
    try:
        import antenv.axon_hooks  # noqa: F401
    except Exception:
        m = types.ModuleType("antenv.axon_hooks")
        m._h = None
        m.set_axon_ntff_profile_hook = lambda h: setattr(m, "_h", h)
        m.get_axon_ntff_profile_hook = lambda: m._h
        sys.modules["antenv.axon_hooks"] = m


_ensure_axon_hooks()

N_CORES = 8
S = 512          # input/output row length
NF = 1024        # padded length
P = 128          # SBUF partitions
KC = S // P      # 4 contraction chunks
OC = S // P      # 4 output-row chunks
BC_PER_CORE = 2  # 16 (b,c) tiles / 8 cores

# Exposed for the test harness: exec time of the last device run (ns), if
# profiling was enabled via BUTTERFLY_TRACE=1.
last_exec_time_ns = None
last_results = None


def _butterfly_np(tw, x, increasing):
    # Mirrors the reference butterfly exactly, in numpy (any dtype).
    B, n = x.shape
    m = tw.shape[0]
    order = range(m) if increasing else range(m - 1, -1, -1)
    for idx in order:
        s = 1 << idx
        t = tw[idx].reshape(n // (2 * s), s, 2, 2)
        xr = x.reshape(B, n // (2 * s), 2, s)
        x = np.einsum('gjik,bgkj->bgij', t, xr).reshape(B, n)
    return x


def _compose_wt(twiddle_fft, twiddle_ifft, fourier_filter_br):
    """Fold twiddles+filter into the lhsT operand Wt[i_in, o_out] (512x512 f64)."""
    tw_fft = np.asarray(twiddle_fft, dtype=np.float64)
    tw_ifft = np.asarray(twiddle_ifft, dtype=np.float64)
    filt = np.asarray(fourier_filter_br, dtype=np.float64)
    tf = tw_fft[0, ..., 0] + 1j * tw_fft[0, ..., 1]
    ti = tw_ifft[0, ..., 0] + 1j * tw_ifft[0, ..., 1]
    X = np.eye(NF, dtype=np.complex128)      # row j = e_j
    X = _butterfly_np(tf, X, increasing=False)
    X = X * filt[None, :]
    X = _butterfly_np(ti, X, increasing=True)
    # X = chain(I) = A^T, so X[i, o] = A[o, i]; W[o, i] = Re(A[o, i]).
    # lhsT for out = lhsT.T @ rhs must be Wt[i, o] = W[o, i] = Re(X[i, o]).
    return np.ascontiguousarray(np.real(X[:S, :S]))


def _mm_dtype():
    name = os.environ.get("BUTTERFLY_MM_DTYPE", "fp16")
    return {
        "fp16": (mybir.dt.float16, np.float16),
        "bf16": (mybir.dt.bfloat16, None),  # needs ml_dtypes; fp16 preferred
    }[name]


def _build_nc():
    # Raw Bass (no TileContext): this walrus encodes at most ONE semaphore
    # wait per instruction, which Tile's scheduler and epilogue drain cannot
    # guarantee. With manual engine programs every wait is its own wait_ge.
    #
    # Layout (per core), all fp16:
    #   p[k] (128, 1024) = [W_k | x0_k]: contraction chunk k of the operator
    #   fused with bc-tile-0's chunk, 256 KiB per DMA piece on the sync
    #   ring, so compute starts on the first piece. x1 (128, 2048) is all
    #   of bc-tile-1, one 512 KiB DMA on the scalar ring issued at t=0.
    #   acc[bc*4+o] accumulates in one PSUM bank per group; DVE copies
    #   PSUM fp32 -> SBUF fp16; ACT stores 128 KiB chunks as each group
    #   completes (bc0's groups finish while bc1 computes).
    mmdt, _ = _mm_dtype()
    f32 = mybir.dt.float32
    n_warm = int(os.environ.get("BUTTERFLY_NWARM", "6"))
    no_gpsimd_drain = os.environ.get("BUTTERFLY_GPSIMD_DRAIN", "0") != "1"

    nc = bass.Bass()
    pd = nc.declare_dram_parameter("p", [KC, P, 2 * S], mmdt, isOutput=False)
    x1d = nc.declare_dram_parameter("x1", [P, KC * S], mmdt, isOutput=False)
    out = nc.declare_dram_parameter("out", [BC_PER_CORE, S, S], mmdt, isOutput=True)

    with ExitStack() as ctx:
        p_sb = [
            ctx.enter_context(nc.sbuf_tensor(f"p_sb{k}", [P, 2 * S], mmdt))
            for k in range(KC)
        ]
        x1_sb = ctx.enter_context(nc.sbuf_tensor("x1_sb", [P, KC * S], mmdt))
        warm_sb = ctx.enter_context(nc.sbuf_tensor("warm_sb", [P, P + S], mmdt))
        o_sb = [
            ctx.enter_context(nc.sbuf_tensor(f"o_sb{j}", [P, OC * S], mmdt))
            for j in range(BC_PER_CORE)
        ]
        accs = [
            ctx.enter_context(nc.psum_tensor(f"acc{g}", [P, S], f32))
            for g in range(BC_PER_CORE * OC)
        ]
        s_p = [ctx.enter_context(nc.semaphore(f"s_p{k}")) for k in range(KC)]
        s_x1 = ctx.enter_context(nc.semaphore("s_x1"))
        s_warm = ctx.enter_context(nc.semaphore("s_warm"))
        s_pe = ctx.enter_context(nc.semaphore("s_pe"))
        s_dve = ctx.enter_context(nc.semaphore("s_dve"))
        s_out = ctx.enter_context(nc.semaphore("s_out"))
        block = ctx.enter_context(nc.Block(no_gpsimd_drain=no_gpsimd_drain))

        @block.sync
        def _(sync):
            # Input pieces, issue order = consumption order, 256 KiB each.
            for k in range(KC):
                sync.dma_start(p_sb[k][:], pd[k]).then_inc(s_p[k], 16)
            # Output stores must have landed in HBM before the NEFF ends;
            # the exit-barrier drain does NOT wait for HWDGE data receipt.
            sync.wait_ge(s_out, BC_PER_CORE * OC * 16)

        @block.scalar
        def _(scalar):
            # x1 on the ACT ring streams in parallel with the sync-ring
            # pieces; it is only needed once bc0's 16 matmuls are done.
            scalar.dma_start(x1_sb[:], x1d[:]).then_inc(s_x1, 16)
            # Per-group 128 KiB stores drain each output chunk as soon as
            # DVE has copied it out of PSUM.
            for g in range(BC_PER_CORE * OC):
                bc, o = divmod(g, OC)
                scalar.wait_ge(s_dve, g + 1)
                scalar.dma_start(
                    out[bc, bass.ts(o, P), :], o_sb[bc][:, bass.ts(o, S)]
                ).then_inc(s_out, 16)

        @block.tensor
        def _(tensor):
            # Warm-up matmuls on a zeroed scratch tile: keeps the PE busy
            # while inputs stream in so HAM un-throttles (1.2 -> 2.4 GHz)
            # before the real matmuls. Results land in acc 7 which is
            # cleared by its real accumulation group's start=True later.
            tensor.wait_ge(s_warm, 1)
            for _ in range(n_warm):
                nc.tensor.matmul(
                    accs[-1][:], warm_sb[:, :P], warm_sb[:, P:],
                    start=True, stop=True,
                )
            # bc0: k-outer so compute starts on the first 256 KiB piece.
            for k in range(KC):
                tensor.wait_ge(s_p[k], 16)
                for o in range(OC):
                    mm = nc.tensor.matmul(
                        accs[o][:],
                        p_sb[k][:, bass.ts(o, P)],
                        p_sb[k][:, S : 2 * S],
                        start=(k == 0),
                        stop=(k == KC - 1),
                    )
                    if k == KC - 1:
                        mm.then_inc(s_pe, 1)
            # bc1: o-outer (group-sequential) so each output group
            # completes 4 matmuls apart and the stores stagger.
            tensor.wait_ge(s_x1, 16)
            for o in range(OC):
                for k in range(KC):
                    mm = nc.tensor.matmul(
                        accs[OC + o][:],
                        p_sb[k][:, bass.ts(o, P)],
                        x1_sb[:, bass.ts(k, S)],
                        start=(k == 0),
                        stop=(k == KC - 1),
                    )
                mm.then_inc(s_pe, 1)

        @block.vector
        def _(vector):
            nc.vector.memset(warm_sb[:], 0.0).then_inc(s_warm, 1)
            for g in range(BC_PER_CORE * OC):
                bc, o = divmod(g, OC)
                vector.wait_ge(s_pe, g + 1)
                nc.vector.tensor_copy(
                    o_sb[bc][:, bass.ts(o, S)], accs[g][:]
                ).then_inc(s_dve, 1)

    return nc


def kernel(x, twiddle_fft, twiddle_ifft, fourier_filter_br):
    global last_exec_time_ns, last_results
    x = np.asarray(x, dtype=np.float32)
    b, c, s_len, a = x.shape
    assert (b, c, s_len, a) == (8, 2, S, S)

    _, npdt = _mm_dtype()
    wt = _compose_wt(twiddle_fft, twiddle_ifft, fourier_filter_br)
    wt4 = wt.reshape(KC, P, S).astype(npdt)
    x16 = x.reshape(b * c, KC, P, S).astype(npdt)  # [bc, k, p, a]

    in_maps = []
    for core in range(N_CORES):
        x0 = x16[BC_PER_CORE * core]
        x1 = x16[BC_PER_CORE * core + 1]
        # p[k] = [w_k | x0_k] along the free dim, one 256 KiB DMA piece each
        pieces = np.concatenate([wt4, x0], axis=2)  # (4, 128, 1024)
        in_maps.append(
            {
                "p": np.ascontiguousarray(pieces),
                # x1 packed [p, k*512+a] so one plain (128, 2048) DMA lands
                # all 4 contraction chunks side by side in SBUF.
                "x1": np.ascontiguousarray(
                    x1.transpose(1, 0, 2).reshape(P, KC * S)
                ),
            }
        )
    nc = _build_nc()
    trace = os.environ.get("BUTTERFLY_TRACE") == "1"
    res = run_bass_kernel_spmd(nc, in_maps, core_ids=list(range(N_CORES)), trace=trace)
    last_exec_time_ns = res.exec_time_ns
    last_results = res

    q = np.concatenate(
        [res.results[k]["out"].astype(np.float32) for k in range(N_CORES)], axis=0
    )
    # q[bc, o, a] = proj.T[o, bc*512 + a]; reference output is
    # proj.T.reshape(b, c, s, a) — a pure reinterpret of the (512, 8192) buffer.
    out = q.transpose(1, 0, 2).reshape(S, b * c * a).reshape(b, c, s_len, a)
    return np.ascontiguousarray(out)
